# revision 4
# baseline (speedup 1.0000x reference)
"""Distributed Trainium2 kernel for the attention GEMV chain:

    score = context_vector @ query            [L]         (L=8192, Q=4096)
    attn  = softmax(score)
    s_t   = attn @ context_vector             [Q]
    out   = K_w @ concat(query, s_t)          [Q]

Sharding over 8 NeuronCores:
  - context_vector rows: 1024 per core (score GEMV + partial weighted sum)
  - K_w rows: 512 per core (each core produces its own slice of the output,
    so no output collective is needed)
  - one AllGather of [s_t_partial(4096), local_max, local_expsum] per core

Per-core compute:
  - score / final GEMVs: fused multiply+reduce (tensor_tensor_reduce) on DVE
  - s_t partial: TensorE matmuls with exp-weights as 1-column stationary
  - softmax cross-core combine: alpha-weighted rank-8 matmul that broadcasts
    the normalized s_t to 128 partitions directly in PSUM
"""
import sys

if "/opt/trn_rl_repo" not in sys.path:
    sys.path.insert(0, "/opt/trn_rl_repo")

from contextlib import ExitStack

import numpy as np

import concourse.bass as bass
import concourse.bacc as bacc
import concourse.mybir as mybir
import concourse.tile as tile
from concourse.bass_isa import ReduceOp
from concourse.bass_utils import run_bass_kernel_spmd

N_CORES = 8
Q = 4096
L = 8192
L_SHARD = L // N_CORES          # 1024 rows of context_vector per core
R_SHARD = Q // N_CORES          # 512 rows of K_w per core
LT = L_SHARD // 128             # 8 l-tiles per core
RT = R_SHARD // 128             # 4 r-tiles per core
NB = Q // 512                   # 8 psum banks of 512 fp32
CCW = Q + 8                     # collective row: partial(4096), max, sum, pad
DT = mybir.dt.float32

_NC_CACHE = {}


def build_nc():
    nc = bacc.Bacc("TRN2", target_bir_lowering=False, debug=False,
                   num_devices=N_CORES)

    q_ext = nc.dram_tensor("query", [1, Q], DT, kind="ExternalInput")
    cv_ext = nc.dram_tensor("cv", [L_SHARD, Q], DT, kind="ExternalInput")
    kw_ext = nc.dram_tensor("kw", [R_SHARD, 2 * Q], DT, kind="ExternalInput")
    out_ext = nc.dram_tensor("out", [128, RT], DT, kind="ExternalOutput")

    cc_in = nc.dram_tensor("cc_in", [1, CCW], DT)
    cc_out = nc.dram_tensor("cc_out", [N_CORES, CCW], DT, addr_space="Shared")

    with tile.TileContext(nc) as tc, ExitStack() as ctx:
        persist = ctx.enter_context(tc.tile_pool(name="persist", bufs=1))
        smalls = ctx.enter_context(tc.tile_pool(name="smalls", bufs=1))

        # query broadcast to all 128 partitions via stride-0 DMA
        queryB = persist.tile([128, Q], DT)
        qa = q_ext.ap()
        q_bcast = bass.AP(tensor=qa.tensor, offset=qa.offset,
                          ap=[[0, 128], list(qa.ap[-1])])
        nc.sync.dma_start(out=queryB, in_=q_bcast)

        scores = smalls.tile([128, LT], DT)
        dummy = smalls.tile([128, 1], DT)

        # ---- phase 1: load cv shard, per-row dot with query -> scores ----
        cv_tiles = []
        with tc.tile_pool(name="cvp", bufs=LT) as cvp:
            for t in range(LT):
                cv_t = cvp.tile([128, Q], DT)
                nc.sync.dma_start(out=cv_t, in_=cv_ext[t * 128:(t + 1) * 128, :])
                nc.vector.scalar_tensor_tensor(
                    out=dummy.broadcast_to([128, Q]),
                    in0=cv_t,
                    scalar=1.0,
                    in1=queryB,
                    op0=mybir.AluOpType.mult,
                    op1=mybir.AluOpType.mult,
                    accum_out=scores[:, t:t + 1],
                )
                cv_tiles.append(cv_t)

            # ---- phase 2: local softmax stats ----
            mcol = smalls.tile([128, 1], DT)
            nc.vector.tensor_reduce(out=mcol, in_=scores,
                                    axis=mybir.AxisListType.X,
                                    op=mybir.AluOpType.max)
            mall = smalls.tile([128, 1], DT)
            nc.gpsimd.partition_all_reduce(mall, mcol, 128, ReduceOp.max)
            negm = smalls.tile([128, 1], DT)
            nc.vector.tensor_scalar_mul(negm, mall, -1.0)
            e = smalls.tile([128, LT], DT)
            nc.scalar.activation(out=e, in_=scores,
                                 func=mybir.ActivationFunctionType.Exp,
                                 bias=negm, scale=1.0)
            sexp = smalls.tile([128, 1], DT)
            nc.vector.tensor_reduce(out=sexp, in_=e,
                                    axis=mybir.AxisListType.X,
                                    op=mybir.AluOpType.add)
            sall = smalls.tile([128, 1], DT)
            nc.gpsimd.partition_all_reduce(sall, sexp, 128, ReduceOp.add)

            # ---- phase 3: unnormalized partial s_t via TensorE ----
            with tc.tile_pool(name="ps1", bufs=1, space="PSUM") as ps1:
                psum_st = ps1.tile([1, Q], DT)
                for n in range(NB):
                    sl = slice(n * 512, (n + 1) * 512)
                    for t in range(LT):
                        nc.tensor.matmul(
                            psum_st[0:1, sl],
                            lhsT=e[:, t:t + 1],
                            rhs=cv_tiles[t][:, sl],
                            start=(t == 0),
                            stop=(t == LT - 1),
                        )
                st_row = persist.tile([1, Q], DT)
                nc.scalar.copy(st_row, psum_st)

        # ---- phase 4: stage + AllGather ----
        nc.sync.dma_start(out=cc_in[0:1, 0:Q], in_=st_row)
        nc.sync.dma_start(out=cc_in[0:1, Q:Q + 1], in_=mall[0:1, 0:1])
        nc.sync.dma_start(out=cc_in[0:1, Q + 1:Q + 2], in_=sall[0:1, 0:1])
        nc.gpsimd.collective_compute(
            "AllGather",
            mybir.AluOpType.bypass,
            replica_groups=[list(range(N_CORES))],
            ins=[cc_in.ap().opt()],
            outs=[cc_out.ap().opt()],
        )
        gathered = persist.tile([N_CORES, CCW], DT)
        nc.sync.dma_start(out=gathered, in_=cc_out.ap())

        # ---- phase 5: global softmax combine + broadcast s_t into PSUM ----
        mg = gathered[:, Q:Q + 1]
        sg = gathered[:, Q + 1:Q + 2]
        mmax = smalls.tile([N_CORES, 1], DT)
        nc.gpsimd.partition_all_reduce(mmax, mg, N_CORES, ReduceOp.max)
        negM = smalls.tile([N_CORES, 1], DT)
        nc.vector.tensor_scalar_mul(negM, mmax, -1.0)
        expm = smalls.tile([N_CORES, 1], DT)
        nc.scalar.activation(out=expm, in_=mg,
                             func=mybir.ActivationFunctionType.Exp,
                             bias=negM, scale=1.0)
        w = smalls.tile([N_CORES, 1], DT)
        nc.vector.tensor_mul(w, expm, sg)
        wsum = smalls.tile([N_CORES, 1], DT)
        nc.gpsimd.partition_all_reduce(wsum, w, N_CORES, ReduceOp.add)
        rS = smalls.tile([N_CORES, 1], DT)
        nc.vector.reciprocal(rS, wsum)
        alpha = smalls.tile([N_CORES, 1], DT)
        nc.vector.tensor_mul(alpha, expm, rS)
        alpha_rep = smalls.tile([N_CORES, 128], DT)
        nc.vector.memset(alpha_rep, 1.0)
        nc.vector.tensor_scalar_mul(alpha_rep, alpha_rep, alpha)

        accq = smalls.tile([128, RT], DT)
        accs = smalls.tile([128, RT], DT)
        acc = smalls.tile([128, RT], DT)

        with tc.tile_pool(name="ps2", bufs=1, space="PSUM") as ps2, \
             tc.tile_pool(name="kwp", bufs=3) as kwp:
            psum_stB = ps2.tile([128, Q], DT)
            for n in range(NB):
                sl = slice(n * 512, (n + 1) * 512)
                nc.tensor.matmul(
                    psum_stB[:, sl],
                    lhsT=alpha_rep,
                    rhs=gathered[0:N_CORES, sl],
                    start=True,
                    stop=True,
                )

            # ---- phase 6: final linear, row-sharded K_w ----
            for j in range(RT):
                kw_j = kwp.tile([128, 2 * Q], DT)
                nc.sync.dma_start(out=kw_j,
                                  in_=kw_ext[j * 128:(j + 1) * 128, :])
                nc.vector.scalar_tensor_tensor(
                    out=dummy.broadcast_to([128, Q]),
                    in0=kw_j[:, 0:Q],
                    scalar=1.0,
                    in1=queryB,
                    op0=mybir.AluOpType.mult,
                    op1=mybir.AluOpType.mult,
                    accum_out=accq[:, j:j + 1],
                )
                nc.vector.scalar_tensor_tensor(
                    out=dummy.broadcast_to([128, Q]),
                    in0=kw_j[:, Q:2 * Q],
                    scalar=1.0,
                    in1=psum_stB,
                    op0=mybir.AluOpType.mult,
                    op1=mybir.AluOpType.mult,
                    accum_out=accs[:, j:j + 1],
                )

        nc.vector.tensor_add(acc, accq, accs)

        nc.sync.dma_start(out=out_ext.ap(), in_=acc)

    nc.compile()
    return nc


def get_nc():
    if "nc" not in _NC_CACHE:
        _NC_CACHE["nc"] = build_nc()
    return _NC_CACHE["nc"]


def _shard_inputs(query, context_vector, K_w):
    q2 = np.ascontiguousarray(query.reshape(1, Q), dtype=np.float32)
    in_maps = []
    for c in range(N_CORES):
        in_maps.append({
            "query": q2,
            "cv": np.ascontiguousarray(
                context_vector[c * L_SHARD:(c + 1) * L_SHARD], dtype=np.float32),
            "kw": np.ascontiguousarray(
                K_w[c * R_SHARD:(c + 1) * R_SHARD], dtype=np.float32),
        })
    return in_maps


def kernel(query, context_vector, K_w, _trace=False, _trace_kwargs=None):
    nc = get_nc()
    in_maps = _shard_inputs(query, context_vector, K_w)
    res = run_bass_kernel_spmd(nc, in_maps, core_ids=list(range(N_CORES)),
                               trace=_trace, **(_trace_kwargs or {}))
    out = np.concatenate(
        [np.asarray(res.results[c]["out"]).T.reshape(-1) for c in range(N_CORES)]
    ).astype(np.float32)
    if _trace:
        kernel.last_results = res
    return out
